# revision 44
# baseline (speedup 1.0000x reference)
"""Trainium2 Bass kernel: dual-softmax cross-attention bilinear forms.

Math (per batch b, a = corr[b] in [N, N], N = 3072):
    attn = exp(2a) * (1/rowsum_a) outer (1/colsum_a)
    fund1 = v1^T attn v1,  fund2^T = v2^T attn^T v2
Device computes, per core (4 batches x 2 row-halves = 8 cores):
    E' = exp(a - 3) fp16 (scalar engine, rowsum via activation accum)
    E2 = E'^2 = exp(2a - 6) fp8e4 (vector; last SQ_SCALAR tiles on the
    scalar engine, which is idle after its exps while the fp8-writing
    DVE multiply only runs at 1x)
    vr = v * e^6 / rowsumE fp8e4,  v = [x1 | x2 | pos]
    X  = E2^T @ vr  -- fp8 DoubleRow matmuls (256-row contraction/pass)
    Xpos^T = vr_pos^T @ E2 -- transposed so the 6-wide runt matmuls are
    not weight-load-bound (moving = 512-elem e2 chunks)
    colsumE partials via ones^T @ E' matmuls into 2 psum banks
Host finishes: colsum normalization + the small [N,262] bilinear GEMMs.

DMA grain: descriptors are generated per SBUF partition row (~0.1us
each), so transfer latency is row-count, not bytes. Hence: a-tiles are
split into 4 row-range sub-DMAs on separate queues; v comes packed
partition-major [128, NT*518] in one 4-way-split load; X accumulates
in SBUF fp16 and leaves as one packed [128, MT*512] 4-way-split store.

PSUM (8 banks): 3 rotating [128,512] X accumulators (x1-cols 0:256
start=True clears the bank, x2-cols 256:512 start=False ride the
per-element has_written bits), 2 colsum banks, 3 pos^T banks (6 chunk
slots of [16,256] at partition 0, pre-zeroed, start=False; 12 chunks
go through in 2 rounds - round 1 replays after the stream).
"""

import numpy as np

import concourse.tile as tile
from concourse import bacc, bass_utils, mybir

B, N, C = 4, 3072, 256
H, W = 48, 64
CP = C + 6          # 262
CX = 2 * C          # 512: [x1 256 | x2 256]; pos 6 separate
CV = CX + 6         # 518
NH = N // 2         # 1536 rows per core
NT = NH // 128      # 12 row tiles per core
NP = NT // 2        # 6 DoubleRow ipairs
MT = N // 128       # 24 column tiles
CS_CHUNK = 512
NCS = N // CS_CHUNK  # 6 colsum psum chunks
NPC = N // 256       # 12 pos^T column chunks
CVP = CX + 16        # 528: fp8 v row: [x1 256 | x2 256 | pos 6 | pad 10]
B_SHIFT = 2.875      # E'' = exp(a + B_SHIFT); constants cancel on host

M0 = 22             # m-tiles whose contraction is split (3,3) ipairs
PAIR_SPLIT = 3      # chunk-0 ipairs (of NP=6) for the split m-tiles
SQ_SCALAR = 1       # trailing tiles whose square runs on the scalar engine

FP32 = mybir.dt.float32
FP16 = mybir.dt.float16
FP8 = mybir.dt.float8e4
DR = mybir.MatmulPerfMode.DoubleRow
MUL = mybir.AluOpType.mult

TRACE = False
LAST_RESULT = None
_CACHED_NC = None


def _build_kernel():
    nc = bacc.Bacc("TRN2", target_bir_lowering=False, debug=False)
    a_in = nc.dram_tensor("a_half", [NH, N], FP16, kind="ExternalInput").ap()
    v_in = nc.dram_tensor("v_half", [128, NT * CVP], FP8, kind="ExternalInput").ap()
    x_out = nc.dram_tensor("x_out", [128, MT * CX], FP16, kind="ExternalOutput").ap()
    pos_out = nc.dram_tensor("pos_out", [16, 2, 3 * CS_CHUNK], FP32, kind="ExternalOutput").ap()
    cs_out = nc.dram_tensor("cs_out", [8, CS_CHUNK], FP32, kind="ExternalOutput").ap()

    with tile.TileContext(nc) as tc:
        _kernel_body(tc, a_in, v_in, x_out, pos_out, cs_out)
    nc.compile()
    return nc


def _pos_slot(ch):
    """pos^T chunk ch (of 12) -> (round, bank t, col offset).

    DoubleRow matmul dst must sit at partition 0, so 6 chunk slots
    (3 banks x 2 col halves) exist; 12 chunks go through in 2 rounds.
    """
    r, s = divmod(ch, 6)
    return r, s // 2, 256 * (s % 2)


def _kernel_body(tc, a_in, v_in, x_out, pos_out, cs_out):
    nc = tc.nc
    with (
        tc.tile_pool(name="singles", bufs=1) as singles,
        tc.tile_pool(name="a_pool", bufs=4) as a_pool,
        tc.tile_pool(name="e_pool", bufs=12) as e_pool,
        tc.tile_pool(name="cs_psum", bufs=1, space="PSUM") as cs_psum,
        tc.tile_pool(name="pos_psum", bufs=1, space="PSUM") as pos_psum,
        tc.tile_pool(name="x_psum", bufs=3, space="PSUM") as x_psum,
    ):
        ones_t = singles.tile([128, 1], FP16)
        nc.vector.memset(ones_t, 1.0)
        ones_f32 = singles.tile([128, 1], FP32)
        nc.vector.memset(ones_f32, 1.0)
        bias_t = singles.tile([128, 1], FP32)
        nc.vector.memset(bias_t, B_SHIFT)

        # prefetch the exp table-set off the critical path
        dummy_t = singles.tile([128, 1], FP32)
        nc.scalar.activation(
            out=dummy_t, in_=bias_t, func=mybir.ActivationFunctionType.Exp
        )

        # vr = fp8(v) arrives pre-packed from the host:
        # [x1 256 | x2 256 | pos 6 | pad 10] per tile (pos padded to a
        # 16-stride so the fp8 ldweights step % 16 == 0 rule holds)
        vr_all = singles.tile([128, NT, CVP], FP8)
        e2_all = singles.tile([128, NT, N], FP8)
        rowsum_all = singles.tile([128, NT], FP32)
        rinv_all = singles.tile([128, NT], FP32)
        rsq_all = singles.tile([128, NT], FP32)
        x_all = singles.tile([128, MT, CX], FP16)



        # 6 colsum chunks packed into 2 psum banks at partitions 0/32/64/96.
        # Pre-zeroed; every matmul accumulates (start=False).
        cs_bank = [
            cs_psum.tile([128, CS_CHUNK], FP32, name=f"csb{t}", tag=f"csb{t}")
            for t in range(2)
        ]
        for t in range(2):
            nc.vector.memset(cs_bank[t], 0.0)

        def cs_ap(j):
            t, p = divmod(j, 4)
            return cs_bank[t][32 * p : 32 * p + 1, :]

        # pos^T chunks: 6 slots of [16, 256] over 3 pre-zeroed banks
        pos_bank = [
            pos_psum.tile([128, CS_CHUNK], FP32, name=f"posb{t}", tag=f"posb{t}")
            for t in range(3)
        ]
        for t in range(3):
            nc.vector.memset(pos_bank[t], 0.0)
        pos_sb = singles.tile([128, 2, 3 * CS_CHUNK], FP32)

        e_pend = [None]

        def flush_square():
            if e_pend[0] is not None:
                i, e_t = e_pend[0]
                nc.vector.scalar_tensor_tensor(
                    out=e2_all[:, i, :],
                    in0=e_t,
                    scalar=rinv_all[:, i : i + 1],
                    in1=e_t,
                    op0=MUL,
                    op1=MUL,
                )
                e_pend[0] = None

        def stream_tile(i):
            a_t = a_pool.tile([128, N], FP16, name="a_t", tag="a_t")
            # one dma_start: its row descriptors round-robin all queues
            nc.sync.dma_start(out=a_t, in_=a_in[i * 128 : (i + 1) * 128, :])
            if i < 4:
                # fp8 v load (packed partition-major) rides in 4 pieces
                # behind the first a-tiles so it never delays the stream
                nc.sync.dma_start(
                    out=vr_all[:, 3 * i : 3 * (i + 1), :],
                    in_=v_in[:, 3 * CVP * i : 3 * CVP * (i + 1)],
                )

            flush_square()

            # E'' = exp(a + B_SHIFT) fp16; rowsum'' via activation accum
            e_t = e_pool.tile([128, N], FP16, name="e_t", tag="e_t")
            nc.scalar.activation(
                out=e_t,
                in_=a_t,
                func=mybir.ActivationFunctionType.Exp,
                bias=bias_t,
                scale=1.0,
                accum_out=rowsum_all[:, i : i + 1],
            )

            # colsum partials: ones^T @ E', accumulated over all tiles
            for j in range(NCS):
                nc.tensor.matmul(
                    cs_ap(j),
                    lhsT=ones_t,
                    rhs=e_t[:, j * CS_CHUNK : (j + 1) * CS_CHUNK],
                    start=False,
                    stop=(i == NT - 1),
                    skip_group_check=True,
                    tile_position=(0, 32 * (j % 4)),
                )

            # E2r = E''^2 / rowsum'' = e^b * exp(2a) / rowsum_a, fp8.
            # The row normalization rides in E2r so vr = fp8(v) verbatim.
            nc.vector.reciprocal(
                rinv_all[:, i : i + 1], rowsum_all[:, i : i + 1]
            )
            if i < NT - SQ_SCALAR:
                e_pend[0] = (i, e_t)
            else:
                # scalar engine: Square(E'' * rsqrt(rowsum''))
                nc.scalar.sqrt(rsq_all[:, i : i + 1], rinv_all[:, i : i + 1])
                nc.scalar.activation(
                    out=e2_all[:, i, :],
                    in_=e_t,
                    func=mybir.ActivationFunctionType.Square,
                    scale=rsq_all[:, i : i + 1],
                )

        def pos_gemm(p, rnd):
            """Xpos^T += vr_pos_pair^T @ e2_pair for round rnd's 6 chunks."""
            lhsT = vr_all[:, 2 * p : 2 * p + 2, CX : CX + 16]
            for ch in range(6 * rnd, 6 * rnd + 6):
                _, t, pcol = _pos_slot(ch)
                nc.tensor.matmul(
                    pos_bank[t][0:16, pcol : pcol + 256],
                    lhsT=lhsT,
                    rhs=e2_all[:, 2 * p : 2 * p + 2, ch * 256 : (ch + 1) * 256],
                    start=False,
                    stop=(p == NP - 1),
                    perf_mode=DR,
                    skip_group_check=True,
                )

        def pos_export(rnd):
            for t in range(3):
                nc.scalar.copy(
                    out=pos_sb[:, rnd, 512 * t : 512 * (t + 1)],
                    in_=pos_bank[t],
                )
                if rnd == 0:
                    nc.vector.memset(pos_bank[t], 0.0)

        def gemm(m, p_lo, p_hi, xp):
            """Accumulate ipairs [p_lo, p_hi) of m's X into psum tile xp."""
            for p in range(p_lo, p_hi):
                lhsT = e2_all[:, 2 * p : 2 * p + 2, m * 128 : (m + 1) * 128]
                first = p == p_lo
                last = p == p_hi - 1
                nc.tensor.matmul(
                    xp[:, 0:256],
                    lhsT=lhsT,
                    rhs=vr_all[:, 2 * p : 2 * p + 2, 0:256],
                    start=first,
                    stop=last,
                    perf_mode=DR,
                    skip_group_check=True,
                )
                nc.tensor.matmul(
                    xp[:, 256:512],
                    lhsT=lhsT,
                    rhs=vr_all[:, 2 * p : 2 * p + 2, 256:512],
                    start=False,  # bank cleared by the 0:256 start
                    stop=last,
                    perf_mode=DR,
                    skip_group_check=True,
                )

        def gemm_c0(m):
            xp = x_psum.tile([128, CX], FP32, name="xp", tag="xp")
            gemm(m, 0, PAIR_SPLIT, xp)
            if m % 2 == 0:
                nc.vector.tensor_copy(out=x_all[:, m, :], in_=xp)
            else:
                nc.scalar.copy(out=x_all[:, m, :], in_=xp)

        def gemm_tail(m):
            """Chunk-1 (for m < M0) or full contraction (m >= M0)."""
            xp = x_psum.tile([128, CX], FP32, name="xp", tag="xp")
            if m < M0:
                gemm(m, PAIR_SPLIT, NP, xp)
                nc.vector.tensor_add(x_all[:, m, :], xp, x_all[:, m, :])
            else:
                gemm(m, 0, NP, xp)
                nc.scalar.copy(out=x_all[:, m, :], in_=xp)

        # ---- phase A: stream chunk-0 tiles; pos^T round 0 fills idle PE ----
        for i in range(2 * PAIR_SPLIT):
            stream_tile(i)
            if i % 2 == 1:
                pos_gemm(i // 2, 0)

        # ---- phase B: stream chunk-1, interleave chunk-0 GEMM ----
        n_tail = NT - 2 * PAIR_SPLIT
        done = 0
        for k, i in enumerate(range(2 * PAIR_SPLIT, NT)):
            stream_tile(i)
            if i % 2 == 1:
                pos_gemm(i // 2, 0)
            want = (k + 1) * M0 // n_tail
            for m in range(done, want):
                gemm_c0(m)
            done = want

        flush_square()

        # ---- colsum psum -> sbuf -> DRAM (4 used rows per bank) ----
        cs_sb = singles.tile([128, 2, CS_CHUNK], FP32)
        for t in range(2):
            nc.scalar.copy(out=cs_sb[:, t, :], in_=cs_bank[t])
            nc.sync.dma_start(
                out=cs_out[4 * t : 4 * t + 4, :], in_=cs_sb[0:128:32, t, :]
            )

        # ---- phase D: full-contraction m's first, then chunk-1 m's ----
        order = list(range(M0, MT)) + list(range(M0))
        # store X in groups of 4 m-tiles as soon as a group completes
        finished = [False] * MT
        grp_stored = [False] * (MT // 4)

        def store_ready():
            for g in range(MT // 4):
                if not grp_stored[g] and all(finished[4 * g : 4 * g + 4]):
                    nc.sync.dma_start(
                        out=x_out[:, 4 * g * CX : 4 * (g + 1) * CX],
                        in_=x_all[:, 4 * g : 4 * (g + 1), :],
                    )
                    grp_stored[g] = True

        for idx, m in enumerate(order):
            gemm_tail(m)
            finished[m] = True
            store_ready()
            # replay pos^T round 1 once round 0 has drained
            if idx == 3:
                pos_export(0)
            if idx == 5:
                for p in range(NP):
                    pos_gemm(p, 1)
        pos_export(1)
        nc.sync.dma_start(out=pos_out, in_=pos_sb[0:16, :, :])


def _positional_encodings():
    ys = np.linspace(-1.0, 1.0, H, dtype=np.float32)
    xs = np.linspace(-1.0, 1.0, W, dtype=np.float32)
    p3 = np.tile(ys, W)
    p4 = np.repeat(xs, H)
    pos = np.stack([p3 * p3, p4 * p4, p3 * p4, p3, p4, np.ones_like(p3)], axis=-1)
    return pos.astype(np.float32)  # [N, 6]


def kernel(x1, x2, corr, W_proj, b_proj):
    global _CACHED_NC, LAST_RESULT
    x1 = np.asarray(x1, dtype=np.float32)
    x2 = np.asarray(x2, dtype=np.float32)
    corr = np.asarray(corr, dtype=np.float32)
    W_proj = np.asarray(W_proj, dtype=np.float32)
    b_proj = np.asarray(b_proj, dtype=np.float32)

    import ml_dtypes

    pos = _positional_encodings()
    a = corr.reshape(B, N, N).astype(np.float16)
    # v = [x1 | x2 | pos | pad] quantized to fp8 on the host (vr = v verbatim
    # since the row normalization rides inside E2r on the device)
    v_all = np.zeros((B, N, CVP), dtype=np.float32)
    v_all[:, :, 0:C] = x1
    v_all[:, :, C : 2 * C] = x2
    v_all[:, :, CX : CX + 6] = np.broadcast_to(pos, (B, N, 6))
    v_all = v_all.astype(ml_dtypes.float8_e4m3)

    if _CACHED_NC is None:
        _CACHED_NC = _build_kernel()
    nc = _CACHED_NC

    in_maps = []
    for b in range(B):
        for h in range(2):
            rows = slice(h * NH, (h + 1) * NH)
            # pack v partition-major: v_packed[p, i*CVP + c] = v[i*128+p, c]
            vp = (
                v_all[b, rows, :]
                .reshape(NT, 128, CVP)
                .transpose(1, 0, 2)
                .reshape(128, NT * CVP)
            )
            in_maps.append(
                {
                    "a_half": np.ascontiguousarray(a[b, rows, :]),
                    "v_half": np.ascontiguousarray(vp),
                }
            )

    res = bass_utils.run_bass_kernel_spmd(
        nc, in_maps, core_ids=list(range(8)), trace=TRACE
    )
    LAST_RESULT = res

    v1 = np.concatenate([x1, np.broadcast_to(pos, (B, N, 6))], axis=2)
    v2 = np.concatenate([x2, np.broadcast_to(pos, (B, N, 6))], axis=2)

    out1 = np.empty((B, CP, C), dtype=np.float32)
    out2 = np.empty((B, CP, C), dtype=np.float32)
    for b in range(B):
        r0, r1 = res.results[2 * b], res.results[2 * b + 1]
        # unpack X: X[m*128+p, c] = x_out[p, m*CX + c]
        X = (
            r0["x_out"].astype(np.float32) + r1["x_out"].astype(np.float32)
        ).reshape(128, MT, CX).transpose(1, 0, 2).reshape(N, CX)
        # decode pos^T chunks: posT[0:6, ch*256:(ch+1)*256] from round slots
        pos_raw = r0["pos_out"] + r1["pos_out"]   # [16, 2, 1536]
        posT = np.empty((6, N), dtype=np.float32)
        for ch in range(NPC):
            r, t, pcol = _pos_slot(ch)
            posT[:, ch * 256 : (ch + 1) * 256] = pos_raw[
                0:6, r, 512 * t + pcol : 512 * t + pcol + 256
            ]
        pos_x = posT.T                             # [N, 6]
        # colsum chunks: rows 0-3 = bank0 chunks 0-3, rows 4-5 = chunks 4-5
        colsum = np.empty(N, dtype=np.float32)
        for j in range(NCS):
            t, p = divmod(j, 4)
            colsum[j * CS_CHUNK : (j + 1) * CS_CHUNK] = (
                r0["cs_out"][4 * t + p] + r1["cs_out"][4 * t + p]
            )
        c = 1.0 / colsum
        vc1 = v1[b] * c[:, None]
        vc2 = v2[b] * c[:, None]
        X1 = np.concatenate([X[:, 0:256], pos_x], axis=1)   # [N, 262]
        X2 = np.concatenate([X[:, 256:512], pos_x], axis=1)
        fund1 = X1.T @ vc1      # [262, 262] = v1^T attn v1
        fund2t = X2.T @ vc2     # = (v2^T attn^T v2)^T
        out1[b] = fund1.T @ W_proj + b_proj
        out2[b] = fund2t @ W_proj + b_proj
    return (out2, out1)


# revision 45
# speedup vs baseline: 1.0448x; 1.0448x over previous
"""Trainium2 Bass kernel: dual-softmax cross-attention bilinear forms.

Math (per batch b, a = corr[b] in [N, N], N = 3072):
    attn = exp(2a) * (1/rowsum_a) outer (1/colsum_a)
    fund1 = v1^T attn v1,  fund2^T = v2^T attn^T v2
Device computes, per core (4 batches x 2 row-halves = 8 cores):
    E' = exp(a - 3) fp16 (scalar engine, rowsum via activation accum)
    E2 = E'^2 = exp(2a - 6) fp8e4 (vector; last SQ_SCALAR tiles on the
    scalar engine, which is idle after its exps while the fp8-writing
    DVE multiply only runs at 1x)
    vr = v * e^6 / rowsumE fp8e4,  v = [x1 | x2 | pos]
    X  = E2^T @ vr  -- fp8 DoubleRow matmuls (256-row contraction/pass)
    Xpos^T = vr_pos^T @ E2 -- transposed so the 6-wide runt matmuls are
    not weight-load-bound (moving = 512-elem e2 chunks)
    colsumE partials via ones^T @ E' matmuls into 2 psum banks
Host finishes: colsum normalization + the small [N,262] bilinear GEMMs.

DMA grain: descriptors are generated per SBUF partition row (~0.1us
each), so transfer latency is row-count, not bytes. Hence: a-tiles are
split into 4 row-range sub-DMAs on separate queues; v comes packed
partition-major [128, NT*518] in one 4-way-split load; X accumulates
in SBUF fp16 and leaves as one packed [128, MT*512] 4-way-split store.

PSUM (8 banks): 3 rotating [128,512] X accumulators (x1-cols 0:256
start=True clears the bank, x2-cols 256:512 start=False ride the
per-element has_written bits), 2 colsum banks, 3 pos^T banks (6 chunk
slots of [16,256] at partition 0, pre-zeroed, start=False; 12 chunks
go through in 2 rounds - round 1 replays after the stream).
"""

import numpy as np

import concourse.tile as tile
from concourse import bacc, bass_utils, mybir

B, N, C = 4, 3072, 256
H, W = 48, 64
CP = C + 6          # 262
CX = 2 * C          # 512: [x1 256 | x2 256]; pos 6 separate
CV = CX + 6         # 518
NH = N // 2         # 1536 rows per core
NT = NH // 128      # 12 row tiles per core
NP = NT // 2        # 6 DoubleRow ipairs
MT = N // 128       # 24 column tiles
CS_CHUNK = 512
NCS = N // CS_CHUNK  # 6 colsum psum chunks
NPC = N // 256       # 12 pos^T column chunks
CVP = CX + 16        # 528: fp8 v row: [x1 256 | x2 256 | pos 6 | pad 10]
B_SHIFT = 2.875      # E'' = exp(a + B_SHIFT); constants cancel on host

M0 = 22             # m-tiles whose contraction is split (3,3) ipairs
PAIR_SPLIT = 3      # chunk-0 ipairs (of NP=6) for the split m-tiles
SQ_SCALAR = 1       # trailing tiles whose square runs on the scalar engine

FP32 = mybir.dt.float32
FP16 = mybir.dt.float16
FP8 = mybir.dt.float8e4
DR = mybir.MatmulPerfMode.DoubleRow
MUL = mybir.AluOpType.mult

TRACE = False
LAST_RESULT = None
_CACHED_NC = None


def _build_kernel():
    nc = bacc.Bacc("TRN2", target_bir_lowering=False, debug=False)
    a_in = nc.dram_tensor("a_half", [NH, N], FP16, kind="ExternalInput").ap()
    v_in = nc.dram_tensor("v_half", [128, NT * CVP], FP8, kind="ExternalInput").ap()
    x_out = nc.dram_tensor("x_out", [128, MT * CX], FP16, kind="ExternalOutput").ap()
    pos_out = nc.dram_tensor("pos_out", [16, 2, 3 * CS_CHUNK], FP32, kind="ExternalOutput").ap()
    cs_out = nc.dram_tensor("cs_out", [8, CS_CHUNK], FP32, kind="ExternalOutput").ap()

    with tile.TileContext(nc) as tc:
        _kernel_body(tc, a_in, v_in, x_out, pos_out, cs_out)
    nc.compile()
    return nc


def _pos_slot(ch):
    """pos^T chunk ch (of 12) -> (round, bank t, col offset).

    DoubleRow matmul dst must sit at partition 0, so 6 chunk slots
    (3 banks x 2 col halves) exist; 12 chunks go through in 2 rounds.
    """
    r, s = divmod(ch, 6)
    return r, s // 2, 256 * (s % 2)


def _kernel_body(tc, a_in, v_in, x_out, pos_out, cs_out):
    nc = tc.nc
    with (
        tc.tile_pool(name="singles", bufs=1) as singles,
        tc.tile_pool(name="a_pool", bufs=4) as a_pool,
        tc.tile_pool(name="e_pool", bufs=12) as e_pool,
        tc.tile_pool(name="cs_psum", bufs=1, space="PSUM") as cs_psum,
        tc.tile_pool(name="pos_psum", bufs=1, space="PSUM") as pos_psum,
        tc.tile_pool(name="x_psum", bufs=3, space="PSUM") as x_psum,
    ):
        ones_t = singles.tile([128, 1], FP16)
        nc.vector.memset(ones_t, 1.0)
        ones_f32 = singles.tile([128, 1], FP32)
        nc.vector.memset(ones_f32, 1.0)
        bias_t = singles.tile([128, 1], FP32)
        nc.vector.memset(bias_t, B_SHIFT)

        # prefetch the exp table-set off the critical path
        dummy_t = singles.tile([128, 1], FP32)
        nc.scalar.activation(
            out=dummy_t, in_=bias_t, func=mybir.ActivationFunctionType.Exp
        )

        # vr = fp8(v) arrives pre-packed from the host:
        # [x1 256 | x2 256 | pos 6 | pad 10] per tile (pos padded to a
        # 16-stride so the fp8 ldweights step % 16 == 0 rule holds)
        vr_all = singles.tile([128, NT, CVP], FP8)
        e2_all = singles.tile([128, NT, N], FP8)
        rowsum_all = singles.tile([128, NT], FP32)
        rinv_all = singles.tile([128, NT], FP32)
        rsq_all = singles.tile([128, NT], FP32)
        x_all = singles.tile([128, MT, CX], FP16)



        # 6 colsum chunks packed into 2 psum banks at partitions 0/32/64/96.
        # Pre-zeroed; every matmul accumulates (start=False).
        cs_bank = [
            cs_psum.tile([128, CS_CHUNK], FP32, name=f"csb{t}", tag=f"csb{t}")
            for t in range(2)
        ]
        for t in range(2):
            nc.vector.memset(cs_bank[t], 0.0)

        def cs_ap(j):
            t, p = divmod(j, 4)
            return cs_bank[t][32 * p : 32 * p + 1, :]

        # pos^T chunks: 6 slots of [16, 256] over 3 pre-zeroed banks
        pos_bank = [
            pos_psum.tile([128, CS_CHUNK], FP32, name=f"posb{t}", tag=f"posb{t}")
            for t in range(3)
        ]
        for t in range(3):
            nc.vector.memset(pos_bank[t], 0.0)
        pos_sb = singles.tile([128, 2, 3 * CS_CHUNK], FP32)

        e_pend = []

        def flush_squares():
            for i, e_t in e_pend:
                nc.vector.scalar_tensor_tensor(
                    out=e2_all[:, i, :],
                    in0=e_t,
                    scalar=rinv_all[:, i : i + 1],
                    in1=e_t,
                    op0=MUL,
                    op1=MUL,
                )
            e_pend.clear()

        def stream_tile(i):
            a_t = a_pool.tile([128, N], FP16, name="a_t", tag="a_t")
            # one dma_start: its row descriptors round-robin all queues
            nc.sync.dma_start(out=a_t, in_=a_in[i * 128 : (i + 1) * 128, :])
            if i < 4:
                # fp8 v load (packed partition-major) rides in 4 pieces
                # behind the first a-tiles so it never delays the stream
                nc.sync.dma_start(
                    out=vr_all[:, 3 * i : 3 * (i + 1), :],
                    in_=v_in[:, 3 * CVP * i : 3 * CVP * (i + 1)],
                )

            # E'' = exp(a + B_SHIFT) fp16; rowsum'' via activation accum
            e_t = e_pool.tile([128, N], FP16, name="e_t", tag="e_t")
            nc.scalar.activation(
                out=e_t,
                in_=a_t,
                func=mybir.ActivationFunctionType.Exp,
                bias=bias_t,
                scale=1.0,
                accum_out=rowsum_all[:, i : i + 1],
            )

            # colsum partials: ones^T @ E', accumulated over all tiles
            for j in range(NCS):
                nc.tensor.matmul(
                    cs_ap(j),
                    lhsT=ones_t,
                    rhs=e_t[:, j * CS_CHUNK : (j + 1) * CS_CHUNK],
                    start=False,
                    stop=(i == NT - 1),
                    skip_group_check=True,
                    tile_position=(0, 32 * (j % 4)),
                )

            # E2r = E''^2 / rowsum'' = e^b * exp(2a) / rowsum_a, fp8.
            # The row normalization rides in E2r so vr = fp8(v) verbatim.
            nc.vector.reciprocal(
                rinv_all[:, i : i + 1], rowsum_all[:, i : i + 1]
            )
            if i < NT - SQ_SCALAR:
                # square emission lags one tile so the next tile's
                # reciprocal is queued ahead of this 3.4us vector op
                e_pend.append((i, e_t))
            else:
                # scalar engine: Square(E'' * rsqrt(rowsum''))
                nc.scalar.sqrt(rsq_all[:, i : i + 1], rinv_all[:, i : i + 1])
                nc.scalar.activation(
                    out=e2_all[:, i, :],
                    in_=e_t,
                    func=mybir.ActivationFunctionType.Square,
                    scale=rsq_all[:, i : i + 1],
                )

        def pos_gemm(p, rnd):
            """Xpos^T += vr_pos_pair^T @ e2_pair for round rnd's 6 chunks."""
            lhsT = vr_all[:, 2 * p : 2 * p + 2, CX : CX + 16]
            for ch in range(6 * rnd, 6 * rnd + 6):
                _, t, pcol = _pos_slot(ch)
                nc.tensor.matmul(
                    pos_bank[t][0:16, pcol : pcol + 256],
                    lhsT=lhsT,
                    rhs=e2_all[:, 2 * p : 2 * p + 2, ch * 256 : (ch + 1) * 256],
                    start=False,
                    stop=(p == NP - 1),
                    perf_mode=DR,
                    skip_group_check=True,
                )

        def pos_export(rnd):
            for t in range(3):
                nc.scalar.copy(
                    out=pos_sb[:, rnd, 512 * t : 512 * (t + 1)],
                    in_=pos_bank[t],
                )
                if rnd == 0:
                    nc.vector.memset(pos_bank[t], 0.0)

        def gemm(m, p_lo, p_hi, xp):
            """Accumulate ipairs [p_lo, p_hi) of m's X into psum tile xp."""
            for p in range(p_lo, p_hi):
                lhsT = e2_all[:, 2 * p : 2 * p + 2, m * 128 : (m + 1) * 128]
                first = p == p_lo
                last = p == p_hi - 1
                nc.tensor.matmul(
                    xp[:, 0:256],
                    lhsT=lhsT,
                    rhs=vr_all[:, 2 * p : 2 * p + 2, 0:256],
                    start=first,
                    stop=last,
                    perf_mode=DR,
                    skip_group_check=True,
                )
                nc.tensor.matmul(
                    xp[:, 256:512],
                    lhsT=lhsT,
                    rhs=vr_all[:, 2 * p : 2 * p + 2, 256:512],
                    start=False,  # bank cleared by the 0:256 start
                    stop=last,
                    perf_mode=DR,
                    skip_group_check=True,
                )

        def gemm_c0(m):
            xp = x_psum.tile([128, CX], FP32, name="xp", tag="xp")
            gemm(m, 0, PAIR_SPLIT, xp)
            if m % 2 == 0:
                nc.vector.tensor_copy(out=x_all[:, m, :], in_=xp)
            else:
                nc.scalar.copy(out=x_all[:, m, :], in_=xp)

        def gemm_tail(m):
            """Chunk-1 (for m < M0) or full contraction (m >= M0)."""
            xp = x_psum.tile([128, CX], FP32, name="xp", tag="xp")
            if m < M0:
                gemm(m, PAIR_SPLIT, NP, xp)
                nc.vector.tensor_add(x_all[:, m, :], xp, x_all[:, m, :])
            else:
                gemm(m, 0, NP, xp)
                nc.scalar.copy(out=x_all[:, m, :], in_=xp)

        # ---- streaming: exp/square pipeline + pos^T round 0 + chunk-0 GEMM
        n_tail = NT - 2 * PAIR_SPLIT
        done = 0
        for i in range(NT):
            stream_tile(i)
            pend = e_pend[:-1] if (e_pend and e_pend[-1][0] == i) else e_pend[:]
            if pend:
                # flush squares of tiles < i (emitted after recip(i))
                for j, e_t in pend:
                    nc.vector.scalar_tensor_tensor(
                        out=e2_all[:, j, :],
                        in0=e_t,
                        scalar=rinv_all[:, j : j + 1],
                        in1=e_t,
                        op0=MUL,
                        op1=MUL,
                    )
                del e_pend[: len(pend)]
            if i % 2 == 1 and i >= 3:
                pos_gemm((i - 1) // 2 - 1, 0)
            if i >= 2 * PAIR_SPLIT:
                k = i - 2 * PAIR_SPLIT
                want = (k + 1) * M0 // n_tail
                for m in range(done, min(want, M0)):
                    gemm_c0(m)
                done = max(done, min(want, M0))

        flush_squares()
        pos_gemm(NP - 1, 0)

        # ---- colsum psum -> sbuf -> DRAM (4 used rows per bank) ----
        cs_sb = singles.tile([128, 2, CS_CHUNK], FP32)
        for t in range(2):
            nc.scalar.copy(out=cs_sb[:, t, :], in_=cs_bank[t])
            nc.sync.dma_start(
                out=cs_out[4 * t : 4 * t + 4, :], in_=cs_sb[0:128:32, t, :]
            )

        # ---- phase D: full-contraction m's first, then chunk-1 m's ----
        order = list(range(M0, MT)) + list(range(M0))
        # store X in groups of 4 m-tiles as soon as a group completes
        finished = [False] * MT
        grp_stored = [False] * (MT // 4)

        def store_ready():
            for g in range(MT // 4):
                if not grp_stored[g] and all(finished[4 * g : 4 * g + 4]):
                    nc.sync.dma_start(
                        out=x_out[:, 4 * g * CX : 4 * (g + 1) * CX],
                        in_=x_all[:, 4 * g : 4 * (g + 1), :],
                    )
                    grp_stored[g] = True

        for idx, m in enumerate(order):
            gemm_tail(m)
            finished[m] = True
            store_ready()
            # replay pos^T round 1 once round 0 has drained
            if idx == 3:
                pos_export(0)
            if idx == 5:
                for p in range(NP):
                    pos_gemm(p, 1)
        pos_export(1)
        nc.sync.dma_start(out=pos_out, in_=pos_sb[0:16, :, :])


def _positional_encodings():
    ys = np.linspace(-1.0, 1.0, H, dtype=np.float32)
    xs = np.linspace(-1.0, 1.0, W, dtype=np.float32)
    p3 = np.tile(ys, W)
    p4 = np.repeat(xs, H)
    pos = np.stack([p3 * p3, p4 * p4, p3 * p4, p3, p4, np.ones_like(p3)], axis=-1)
    return pos.astype(np.float32)  # [N, 6]


def kernel(x1, x2, corr, W_proj, b_proj):
    global _CACHED_NC, LAST_RESULT
    x1 = np.asarray(x1, dtype=np.float32)
    x2 = np.asarray(x2, dtype=np.float32)
    corr = np.asarray(corr, dtype=np.float32)
    W_proj = np.asarray(W_proj, dtype=np.float32)
    b_proj = np.asarray(b_proj, dtype=np.float32)

    import ml_dtypes

    pos = _positional_encodings()
    a = corr.reshape(B, N, N).astype(np.float16)
    # v = [x1 | x2 | pos | pad] quantized to fp8 on the host (vr = v verbatim
    # since the row normalization rides inside E2r on the device)
    v_all = np.zeros((B, N, CVP), dtype=np.float32)
    v_all[:, :, 0:C] = x1
    v_all[:, :, C : 2 * C] = x2
    v_all[:, :, CX : CX + 6] = np.broadcast_to(pos, (B, N, 6))
    v_all = v_all.astype(ml_dtypes.float8_e4m3)

    if _CACHED_NC is None:
        _CACHED_NC = _build_kernel()
    nc = _CACHED_NC

    in_maps = []
    for b in range(B):
        for h in range(2):
            rows = slice(h * NH, (h + 1) * NH)
            # pack v partition-major: v_packed[p, i*CVP + c] = v[i*128+p, c]
            vp = (
                v_all[b, rows, :]
                .reshape(NT, 128, CVP)
                .transpose(1, 0, 2)
                .reshape(128, NT * CVP)
            )
            in_maps.append(
                {
                    "a_half": np.ascontiguousarray(a[b, rows, :]),
                    "v_half": np.ascontiguousarray(vp),
                }
            )

    res = bass_utils.run_bass_kernel_spmd(
        nc, in_maps, core_ids=list(range(8)), trace=TRACE
    )
    LAST_RESULT = res

    v1 = np.concatenate([x1, np.broadcast_to(pos, (B, N, 6))], axis=2)
    v2 = np.concatenate([x2, np.broadcast_to(pos, (B, N, 6))], axis=2)

    out1 = np.empty((B, CP, C), dtype=np.float32)
    out2 = np.empty((B, CP, C), dtype=np.float32)
    for b in range(B):
        r0, r1 = res.results[2 * b], res.results[2 * b + 1]
        # unpack X: X[m*128+p, c] = x_out[p, m*CX + c]
        X = (
            r0["x_out"].astype(np.float32) + r1["x_out"].astype(np.float32)
        ).reshape(128, MT, CX).transpose(1, 0, 2).reshape(N, CX)
        # decode pos^T chunks: posT[0:6, ch*256:(ch+1)*256] from round slots
        pos_raw = r0["pos_out"] + r1["pos_out"]   # [16, 2, 1536]
        posT = np.empty((6, N), dtype=np.float32)
        for ch in range(NPC):
            r, t, pcol = _pos_slot(ch)
            posT[:, ch * 256 : (ch + 1) * 256] = pos_raw[
                0:6, r, 512 * t + pcol : 512 * t + pcol + 256
            ]
        pos_x = posT.T                             # [N, 6]
        # colsum chunks: rows 0-3 = bank0 chunks 0-3, rows 4-5 = chunks 4-5
        colsum = np.empty(N, dtype=np.float32)
        for j in range(NCS):
            t, p = divmod(j, 4)
            colsum[j * CS_CHUNK : (j + 1) * CS_CHUNK] = (
                r0["cs_out"][4 * t + p] + r1["cs_out"][4 * t + p]
            )
        c = 1.0 / colsum
        vc1 = v1[b] * c[:, None]
        vc2 = v2[b] * c[:, None]
        X1 = np.concatenate([X[:, 0:256], pos_x], axis=1)   # [N, 262]
        X2 = np.concatenate([X[:, 256:512], pos_x], axis=1)
        fund1 = X1.T @ vc1      # [262, 262] = v1^T attn v1
        fund2t = X2.T @ vc2     # = (v2^T attn^T v2)^T
        out1[b] = fund1.T @ W_proj + b_proj
        out2[b] = fund2t @ W_proj + b_proj
    return (out2, out1)


# revision 46
# speedup vs baseline: 1.0791x; 1.0329x over previous
"""Trainium2 Bass kernel: dual-softmax cross-attention bilinear forms.

Math (per batch b, a = corr[b] in [N, N], N = 3072):
    attn = exp(2a) * (1/rowsum_a) outer (1/colsum_a)
    fund1 = v1^T attn v1,  fund2^T = v2^T attn^T v2
Device computes, per core (4 batches x 2 row-halves = 8 cores):
    E' = exp(a - 3) fp16 (scalar engine, rowsum via activation accum)
    E2 = E'^2 = exp(2a - 6) fp8e4 (vector; last SQ_SCALAR tiles on the
    scalar engine, which is idle after its exps while the fp8-writing
    DVE multiply only runs at 1x)
    vr = v * e^6 / rowsumE fp8e4,  v = [x1 | x2 | pos]
    X  = E2^T @ vr  -- fp8 DoubleRow matmuls (256-row contraction/pass)
    Xpos^T = vr_pos^T @ E2 -- transposed so the 6-wide runt matmuls are
    not weight-load-bound (moving = 512-elem e2 chunks)
    colsumE partials via ones^T @ E' matmuls into 2 psum banks
Host finishes: colsum normalization + the small [N,262] bilinear GEMMs.

DMA grain: descriptors are generated per SBUF partition row (~0.1us
each), so transfer latency is row-count, not bytes. Hence: a-tiles are
split into 4 row-range sub-DMAs on separate queues; v comes packed
partition-major [128, NT*518] in one 4-way-split load; X accumulates
in SBUF fp16 and leaves as one packed [128, MT*512] 4-way-split store.

PSUM (8 banks): 3 rotating [128,512] X accumulators (x1-cols 0:256
start=True clears the bank, x2-cols 256:512 start=False ride the
per-element has_written bits), 2 colsum banks, 3 pos^T banks (6 chunk
slots of [16,256] at partition 0, pre-zeroed, start=False; 12 chunks
go through in 2 rounds - round 1 replays after the stream).
"""

import numpy as np

import concourse.tile as tile
from concourse import bacc, bass_utils, mybir

B, N, C = 4, 3072, 256
H, W = 48, 64
CP = C + 6          # 262
CX = 2 * C          # 512: [x1 256 | x2 256]; pos 6 separate
CV = CX + 6         # 518
NH = N // 2         # 1536 rows per core
NT = NH // 128      # 12 row tiles per core
NP = NT // 2        # 6 DoubleRow ipairs
MT = N // 128       # 24 column tiles
CS_CHUNK = 512
NCS = N // CS_CHUNK  # 6 colsum psum chunks
NPC = N // 256       # 12 pos^T column chunks
CVP = CX + 16        # 528: fp8 v row: [x1 256 | x2 256 | pos 6 | pad 10]
B_SHIFT = 2.875      # E'' = exp(a + B_SHIFT); constants cancel on host

M0 = 20             # m-tiles whose contraction is split (3,3) ipairs
PAIR_SPLIT = 3      # chunk-0 ipairs (of NP=6) for the split m-tiles
SQ_SCALAR = 1       # trailing tiles whose square runs on the scalar engine

FP32 = mybir.dt.float32
FP16 = mybir.dt.float16
FP8 = mybir.dt.float8e4
DR = mybir.MatmulPerfMode.DoubleRow
MUL = mybir.AluOpType.mult

TRACE = False
LAST_RESULT = None
_CACHED_NC = None


def _build_kernel():
    nc = bacc.Bacc("TRN2", target_bir_lowering=False, debug=False)
    a_in = nc.dram_tensor("a_half", [NH, N], FP16, kind="ExternalInput").ap()
    v_in = nc.dram_tensor("v_half", [128, NT * CVP], FP8, kind="ExternalInput").ap()
    x_out = nc.dram_tensor("x_out", [128, MT * CX], FP16, kind="ExternalOutput").ap()
    pos_out = nc.dram_tensor("pos_out", [16, 2, 3 * CS_CHUNK], FP32, kind="ExternalOutput").ap()
    cs_out = nc.dram_tensor("cs_out", [8, CS_CHUNK], FP32, kind="ExternalOutput").ap()

    with tile.TileContext(nc) as tc:
        _kernel_body(tc, a_in, v_in, x_out, pos_out, cs_out)
    nc.compile()
    return nc


def _pos_slot(ch):
    """pos^T chunk ch (of 12) -> (round, bank t, col offset).

    DoubleRow matmul dst must sit at partition 0, so 6 chunk slots
    (3 banks x 2 col halves) exist; 12 chunks go through in 2 rounds.
    """
    r, s = divmod(ch, 6)
    return r, s // 2, 256 * (s % 2)


def _kernel_body(tc, a_in, v_in, x_out, pos_out, cs_out):
    nc = tc.nc
    with (
        tc.tile_pool(name="singles", bufs=1) as singles,
        tc.tile_pool(name="a_pool", bufs=4) as a_pool,
        tc.tile_pool(name="e_pool", bufs=12) as e_pool,
        tc.tile_pool(name="cs_psum", bufs=1, space="PSUM") as cs_psum,
        tc.tile_pool(name="pos_psum", bufs=1, space="PSUM") as pos_psum,
        tc.tile_pool(name="x_psum", bufs=3, space="PSUM") as x_psum,
    ):
        ones_t = singles.tile([128, 1], FP16)
        nc.vector.memset(ones_t, 1.0)
        ones_f32 = singles.tile([128, 1], FP32)
        nc.vector.memset(ones_f32, 1.0)
        bias_t = singles.tile([128, 1], FP32)
        nc.vector.memset(bias_t, B_SHIFT)

        # prefetch the exp table-set off the critical path
        dummy_t = singles.tile([128, 1], FP32)
        nc.scalar.activation(
            out=dummy_t, in_=bias_t, func=mybir.ActivationFunctionType.Exp
        )

        # vr = fp8(v) arrives pre-packed from the host:
        # [x1 256 | x2 256 | pos 6 | pad 10] per tile (pos padded to a
        # 16-stride so the fp8 ldweights step % 16 == 0 rule holds)
        vr_all = singles.tile([128, NT, CVP], FP8)
        e2_all = singles.tile([128, NT, N], FP8)
        rowsum_all = singles.tile([128, NT], FP32)
        rinv_all = singles.tile([128, NT], FP32)
        rsq_all = singles.tile([128, NT], FP32)
        x_all = singles.tile([128, MT, CX], FP16)



        # 6 colsum chunks packed into 2 psum banks at partitions 0/32/64/96.
        # Pre-zeroed; every matmul accumulates (start=False).
        cs_bank = [
            cs_psum.tile([128, CS_CHUNK], FP32, name=f"csb{t}", tag=f"csb{t}")
            for t in range(2)
        ]
        for t in range(2):
            nc.vector.memset(cs_bank[t], 0.0)

        def cs_ap(j):
            t, p = divmod(j, 4)
            return cs_bank[t][32 * p : 32 * p + 1, :]

        # pos^T chunks: 6 slots of [16, 256] over 3 pre-zeroed banks
        pos_bank = [
            pos_psum.tile([128, CS_CHUNK], FP32, name=f"posb{t}", tag=f"posb{t}")
            for t in range(3)
        ]
        for t in range(3):
            nc.vector.memset(pos_bank[t], 0.0)
        pos_sb = singles.tile([128, 2, 3 * CS_CHUNK], FP32)

        e_pend = []

        def flush_squares():
            for i, e_t in e_pend:
                nc.vector.scalar_tensor_tensor(
                    out=e2_all[:, i, :],
                    in0=e_t,
                    scalar=rinv_all[:, i : i + 1],
                    in1=e_t,
                    op0=MUL,
                    op1=MUL,
                )
            e_pend.clear()

        def stream_tile(i):
            a_t = a_pool.tile([128, N], FP16, name="a_t", tag="a_t")
            # one dma_start: its row descriptors round-robin all queues
            nc.sync.dma_start(out=a_t, in_=a_in[i * 128 : (i + 1) * 128, :])
            if i == 0:
                # fp8 v load (packed partition-major) rides behind the
                # first a-tile so it never delays the exp stream
                nc.sync.dma_start(out=vr_all, in_=v_in)

            # E'' = exp(a + B_SHIFT) fp16; rowsum'' via activation accum
            e_t = e_pool.tile([128, N], FP16, name="e_t", tag="e_t")
            nc.scalar.activation(
                out=e_t,
                in_=a_t,
                func=mybir.ActivationFunctionType.Exp,
                bias=bias_t,
                scale=1.0,
                accum_out=rowsum_all[:, i : i + 1],
            )

            # colsum partials: ones^T @ E', accumulated over all tiles
            for j in range(NCS):
                nc.tensor.matmul(
                    cs_ap(j),
                    lhsT=ones_t,
                    rhs=e_t[:, j * CS_CHUNK : (j + 1) * CS_CHUNK],
                    start=False,
                    stop=(i == NT - 1),
                    skip_group_check=True,
                    tile_position=(0, 32 * (j % 4)),
                )

            # E2r = E''^2 / rowsum'' = e^b * exp(2a) / rowsum_a, fp8.
            # The row normalization rides in E2r so vr = fp8(v) verbatim.
            nc.vector.reciprocal(
                rinv_all[:, i : i + 1], rowsum_all[:, i : i + 1]
            )
            if i < NT - SQ_SCALAR:
                # square emission lags one tile so the next tile's
                # reciprocal is queued ahead of this 3.4us vector op
                e_pend.append((i, e_t))
            else:
                # scalar engine: Square(E'' * rsqrt(rowsum''))
                nc.scalar.sqrt(rsq_all[:, i : i + 1], rinv_all[:, i : i + 1])
                nc.scalar.activation(
                    out=e2_all[:, i, :],
                    in_=e_t,
                    func=mybir.ActivationFunctionType.Square,
                    scale=rsq_all[:, i : i + 1],
                )

        def pos_gemm(p, rnd):
            """Xpos^T += vr_pos_pair^T @ e2_pair for round rnd's 6 chunks."""
            lhsT = vr_all[:, 2 * p : 2 * p + 2, CX : CX + 16]
            for ch in range(6 * rnd, 6 * rnd + 6):
                _, t, pcol = _pos_slot(ch)
                nc.tensor.matmul(
                    pos_bank[t][0:16, pcol : pcol + 256],
                    lhsT=lhsT,
                    rhs=e2_all[:, 2 * p : 2 * p + 2, ch * 256 : (ch + 1) * 256],
                    start=False,
                    stop=(p == NP - 1),
                    perf_mode=DR,
                    skip_group_check=True,
                )

        def pos_export(rnd):
            for t in range(3):
                nc.scalar.copy(
                    out=pos_sb[:, rnd, 512 * t : 512 * (t + 1)],
                    in_=pos_bank[t],
                )
                if rnd == 0:
                    nc.vector.memset(pos_bank[t], 0.0)

        def gemm(m, p_lo, p_hi, xp):
            """Accumulate ipairs [p_lo, p_hi) of m's X into psum tile xp."""
            for p in range(p_lo, p_hi):
                lhsT = e2_all[:, 2 * p : 2 * p + 2, m * 128 : (m + 1) * 128]
                first = p == p_lo
                last = p == p_hi - 1
                nc.tensor.matmul(
                    xp[:, 0:256],
                    lhsT=lhsT,
                    rhs=vr_all[:, 2 * p : 2 * p + 2, 0:256],
                    start=first,
                    stop=last,
                    perf_mode=DR,
                    skip_group_check=True,
                )
                nc.tensor.matmul(
                    xp[:, 256:512],
                    lhsT=lhsT,
                    rhs=vr_all[:, 2 * p : 2 * p + 2, 256:512],
                    start=False,  # bank cleared by the 0:256 start
                    stop=last,
                    perf_mode=DR,
                    skip_group_check=True,
                )

        def gemm_c0(m):
            xp = x_psum.tile([128, CX], FP32, name="xp", tag="xp")
            gemm(m, 0, PAIR_SPLIT, xp)
            if m % 2 == 0:
                nc.vector.tensor_copy(out=x_all[:, m, :], in_=xp)
            else:
                nc.scalar.copy(out=x_all[:, m, :], in_=xp)

        def gemm_tail(m):
            """Chunk-1 (for m < M0) or full contraction (m >= M0)."""
            xp = x_psum.tile([128, CX], FP32, name="xp", tag="xp")
            if m < M0:
                gemm(m, PAIR_SPLIT, NP, xp)
                nc.vector.tensor_add(x_all[:, m, :], xp, x_all[:, m, :])
            else:
                gemm(m, 0, NP, xp)
                nc.scalar.copy(out=x_all[:, m, :], in_=xp)

        # ---- streaming: exp/square pipeline + pos^T round 0 + chunk-0 GEMM
        n_tail = NT - 2 * PAIR_SPLIT
        done = 0
        for i in range(NT):
            stream_tile(i)
            # flush squares of tiles <= i-2: emission lags two tiles so
            # the trailing reciprocals queue ahead of the vector backlog
            pend = [pe for pe in e_pend if pe[0] <= i - 2]
            if pend:
                for j, e_t in pend:
                    nc.vector.scalar_tensor_tensor(
                        out=e2_all[:, j, :],
                        in0=e_t,
                        scalar=rinv_all[:, j : j + 1],
                        in1=e_t,
                        op0=MUL,
                        op1=MUL,
                    )
                del e_pend[: len(pend)]
            if i % 2 == 1 and i >= 3:
                pos_gemm((i - 1) // 2 - 1, 0)
            if i >= 2 * PAIR_SPLIT:
                k = i - 2 * PAIR_SPLIT
                want = (k + 1) * M0 // n_tail
                for m in range(done, min(want, M0)):
                    gemm_c0(m)
                done = max(done, min(want, M0))

        flush_squares()
        pos_gemm(NP - 1, 0)

        # ---- colsum psum -> sbuf -> DRAM (4 used rows per bank) ----
        cs_sb = singles.tile([128, 2, CS_CHUNK], FP32)
        for t in range(2):
            nc.scalar.copy(out=cs_sb[:, t, :], in_=cs_bank[t])
            nc.sync.dma_start(
                out=cs_out[4 * t : 4 * t + 4, :], in_=cs_sb[0:128:32, t, :]
            )

        # ---- phase D: full-contraction m's first, then chunk-1 m's ----
        order = list(range(M0, MT)) + list(range(M0))
        # store X in groups of 4 m-tiles as soon as a group completes
        finished = [False] * MT
        grp_stored = [False] * (MT // 4)

        def store_ready():
            for g in range(MT // 4):
                if not grp_stored[g] and all(finished[4 * g : 4 * g + 4]):
                    nc.sync.dma_start(
                        out=x_out[:, 4 * g * CX : 4 * (g + 1) * CX],
                        in_=x_all[:, 4 * g : 4 * (g + 1), :],
                    )
                    grp_stored[g] = True

        for idx, m in enumerate(order):
            gemm_tail(m)
            finished[m] = True
            store_ready()
            # replay pos^T round 1 once round 0 has drained
            if idx == 3:
                pos_export(0)
            if idx == 5:
                for p in range(NP):
                    pos_gemm(p, 1)
        pos_export(1)
        nc.sync.dma_start(out=pos_out, in_=pos_sb[0:16, :, :])


def _positional_encodings():
    ys = np.linspace(-1.0, 1.0, H, dtype=np.float32)
    xs = np.linspace(-1.0, 1.0, W, dtype=np.float32)
    p3 = np.tile(ys, W)
    p4 = np.repeat(xs, H)
    pos = np.stack([p3 * p3, p4 * p4, p3 * p4, p3, p4, np.ones_like(p3)], axis=-1)
    return pos.astype(np.float32)  # [N, 6]


def kernel(x1, x2, corr, W_proj, b_proj):
    global _CACHED_NC, LAST_RESULT
    x1 = np.asarray(x1, dtype=np.float32)
    x2 = np.asarray(x2, dtype=np.float32)
    corr = np.asarray(corr, dtype=np.float32)
    W_proj = np.asarray(W_proj, dtype=np.float32)
    b_proj = np.asarray(b_proj, dtype=np.float32)

    import ml_dtypes

    pos = _positional_encodings()
    a = corr.reshape(B, N, N).astype(np.float16)
    # v = [x1 | x2 | pos | pad] quantized to fp8 on the host (vr = v verbatim
    # since the row normalization rides inside E2r on the device)
    v_all = np.zeros((B, N, CVP), dtype=np.float32)
    v_all[:, :, 0:C] = x1
    v_all[:, :, C : 2 * C] = x2
    v_all[:, :, CX : CX + 6] = np.broadcast_to(pos, (B, N, 6))
    v_all = v_all.astype(ml_dtypes.float8_e4m3)

    if _CACHED_NC is None:
        _CACHED_NC = _build_kernel()
    nc = _CACHED_NC

    in_maps = []
    for b in range(B):
        for h in range(2):
            rows = slice(h * NH, (h + 1) * NH)
            # pack v partition-major: v_packed[p, i*CVP + c] = v[i*128+p, c]
            vp = (
                v_all[b, rows, :]
                .reshape(NT, 128, CVP)
                .transpose(1, 0, 2)
                .reshape(128, NT * CVP)
            )
            in_maps.append(
                {
                    "a_half": np.ascontiguousarray(a[b, rows, :]),
                    "v_half": np.ascontiguousarray(vp),
                }
            )

    res = bass_utils.run_bass_kernel_spmd(
        nc, in_maps, core_ids=list(range(8)), trace=TRACE
    )
    LAST_RESULT = res

    v1 = np.concatenate([x1, np.broadcast_to(pos, (B, N, 6))], axis=2)
    v2 = np.concatenate([x2, np.broadcast_to(pos, (B, N, 6))], axis=2)

    out1 = np.empty((B, CP, C), dtype=np.float32)
    out2 = np.empty((B, CP, C), dtype=np.float32)
    for b in range(B):
        r0, r1 = res.results[2 * b], res.results[2 * b + 1]
        # unpack X: X[m*128+p, c] = x_out[p, m*CX + c]
        X = (
            r0["x_out"].astype(np.float32) + r1["x_out"].astype(np.float32)
        ).reshape(128, MT, CX).transpose(1, 0, 2).reshape(N, CX)
        # decode pos^T chunks: posT[0:6, ch*256:(ch+1)*256] from round slots
        pos_raw = r0["pos_out"] + r1["pos_out"]   # [16, 2, 1536]
        posT = np.empty((6, N), dtype=np.float32)
        for ch in range(NPC):
            r, t, pcol = _pos_slot(ch)
            posT[:, ch * 256 : (ch + 1) * 256] = pos_raw[
                0:6, r, 512 * t + pcol : 512 * t + pcol + 256
            ]
        pos_x = posT.T                             # [N, 6]
        # colsum chunks: rows 0-3 = bank0 chunks 0-3, rows 4-5 = chunks 4-5
        colsum = np.empty(N, dtype=np.float32)
        for j in range(NCS):
            t, p = divmod(j, 4)
            colsum[j * CS_CHUNK : (j + 1) * CS_CHUNK] = (
                r0["cs_out"][4 * t + p] + r1["cs_out"][4 * t + p]
            )
        c = 1.0 / colsum
        vc1 = v1[b] * c[:, None]
        vc2 = v2[b] * c[:, None]
        X1 = np.concatenate([X[:, 0:256], pos_x], axis=1)   # [N, 262]
        X2 = np.concatenate([X[:, 256:512], pos_x], axis=1)
        fund1 = X1.T @ vc1      # [262, 262] = v1^T attn v1
        fund2t = X2.T @ vc2     # = (v2^T attn^T v2)^T
        out1[b] = fund1.T @ W_proj + b_proj
        out2[b] = fund2t @ W_proj + b_proj
    return (out2, out1)
